# revision 11
# baseline (speedup 1.0000x reference)
"""Differentiable Bezier path renderer on 8 Trainium2 NeuronCores.

Strategy
--------
The reference rasterizes M=2048 path edges into a 512x512 soft
winding-number image:

    wind[h, w] = sum_e coeff(e, h) * sigmoid(x_cross(e, h) - w)
    coeff(e,h) = sigmoid(20 t) * sigmoid(20 (1 - t)) * sign(dy_e) * [|dy_e|>=1e-6]
    t          = (h - y0_e) / (dy_e + 1e-8),  x_cross = x0_e + t * dx_e

Two sparsity facts carry the kernel:
  * coeff is negligible (< 5e-8) outside t in [-0.85, 1.85], so only
    ~55k of the 1M (edge, row) pairs matter.
  * sigmoid(x_cross - w) saturates outside |x_cross - w| <= 18, so per
    pair only a 36px-wide transition window needs real sigmoids; the
    whole region left of the window contributes exactly coeff.

The host enumerates active pairs, assigns rows to cores so every core
gets an equal pair load (64 rows per core, no collectives needed), and
packs pairs into blocks of 128 "slots".  Each pair gets one 128-wide,
64-aligned window segment s (columns [64 s, 64 s + 128)) that is
guaranteed to contain its transition; blocks group pairs of the same s.

Per block the device computes, slots on the partition axis:
  * ScalarE : SIGW[p, k] = sigmoid((xc_p - 64 s) - k), k in [0,128)
  * VectorE : W2[p, r] = (iota_r == row_p) * coeff_p     (fused is_eq*mul)
  * TensorE : PSW[r, s-slice] += W2.T @ SIGW             (window part)
              PSL[r, b]       += W2.T @ LMASK            (saturated part,
                LMASK[p, b] = [64 (b+1) <= 64 s_p], b in [0,7))
Afterwards VectorE folds the 8 overlapping stream slices plus the
broadcast left-constants into wind[64, 512], and ScalarE writes
alpha = sigmoid(4 wind) into an interleaved RGBA tile (rgb = broadcast
input color).  The host only gathers per-edge scalars per pair and
reassembles the 8 per-core row sets.
"""

import numpy as np

import concourse.bacc as bacc
import concourse.mybir as mybir
import concourse.tile as tile
from concourse.bass_utils import run_bass_kernel_spmd

H = 512
W = 512
S = 64          # cubic bezier segments
TSAMP = 32      # samples per segment
M = S * TSAMP   # path points == edges
NCORES = 8
RPC = H // NCORES  # rows per core
NSTREAM = 8        # 64-aligned window segment streams
WIN = 18.0         # sigmoid saturation half-width (sigmoid(-18) ~ 1.5e-8)
TB = np.float32(0.85)     # t-window bound: sigmoid(-17) ~ 4.1e-8
CLAMP_T = 60.0            # |20 t| <= 1200, keeps ACT inputs finite
CLAMP_X = 10000.0         # sigmoid saturated way before +-CLAMP_X
DT = mybir.dt.float32
AF = mybir.ActivationFunctionType

_prog_cache = {}


def _host_prep(control_points):
    """Sample the path, enumerate active (edge, row) pairs, assign rows to
    cores, pack pairs into per-stream blocks of 128 slots.

    Returns (per_core_inputs, core_rows, stream_blocks) where
    stream_blocks[s] is the number of blocks of stream s (same for all
    cores; short cores are padded with coeff=0 slots)."""
    cp = np.asarray(control_points, dtype=np.float32)
    p0 = cp[0:3 * S:3][:, None, :]
    p1 = cp[1:3 * S:3][:, None, :]
    p2 = cp[2:3 * S:3][:, None, :]
    p3 = cp[3:3 * S + 1:3][:, None, :]
    t = np.linspace(0.0, 1.0, TSAMP, dtype=np.float32)[None, :, None]
    mt = np.float32(1.0) - t
    pts = (mt ** 3) * p0 + 3.0 * (mt ** 2) * t * p1 \
        + 3.0 * mt * (t ** 2) * p2 + (t ** 3) * p3
    path = pts.reshape(-1, 2).astype(np.float32)

    nxt = np.roll(path, -1, axis=0)
    x0 = path[:, 0]
    y0 = path[:, 1]
    dy = nxt[:, 1] - y0
    dxe = nxt[:, 0] - x0
    dys = (dy + np.float32(1e-8)).astype(np.float32)
    recip = (np.float32(1.0) / dys).astype(np.float32)
    sm = (np.sign(dy) * (np.abs(dy) >= np.float32(1e-6))).astype(np.float32)

    g1 = y0 + (-TB) * dys
    g2 = y0 + (np.float32(1.0) + TB) * dys
    rlo = np.maximum(np.ceil(np.minimum(g1, g2)), 0.0).astype(np.int64)
    rhi = np.minimum(np.floor(np.maximum(g1, g2)), H - 1).astype(np.int64)
    act = (sm != 0) & (rhi >= rlo)
    eact = np.nonzero(act)[0]
    counts = (rhi[eact] - rlo[eact] + 1).astype(np.int64)
    pair_edge = np.repeat(eact, counts)
    pair_row = np.concatenate(
        [np.arange(rlo[e], rhi[e] + 1, dtype=np.int64) for e in eact]
    ) if len(eact) else np.zeros(0, np.int64)

    # Window segment per pair, from host-side x_cross (the ~1 ulp
    # host/device difference is covered by the 64 - 36 px fit margin).
    tval = ((pair_row.astype(np.float32) - y0[pair_edge]) * recip[pair_edge])
    xcv = x0[pair_edge] + tval * dxe[pair_edge]
    xcv = np.clip(xcv, -CLAMP_X, CLAMP_X)
    seg = np.clip(np.floor((xcv - WIN) / 64.0), 0, NSTREAM - 1).astype(np.int64)

    # Balanced row -> core assignment (equal pair load, RPC rows per core).
    rowcnt = np.bincount(pair_row, minlength=H)
    order = np.argsort(-rowcnt, kind="stable")
    core_rows = [[] for _ in range(NCORES)]
    loads = np.zeros(NCORES, np.int64)
    for r in order:
        avail = [c for c in range(NCORES) if len(core_rows[c]) < RPC]
        c = min(avail, key=lambda i: loads[i])
        core_rows[c].append(int(r))
        loads[c] += rowcnt[r]
    row_core = np.empty(H, np.int64)
    row_loc = np.empty(H, np.int64)
    for c in range(NCORES):
        for i, r in enumerate(core_rows[c]):
            row_core[r] = c
            row_loc[r] = i

    pair_core = row_core[pair_row]
    # blocks per stream = max over cores (SPMD: one program for all cores),
    # rounded up so near-identical inputs reuse the compiled program.
    stream_blocks = []
    for s in range(NSTREAM):
        ns = np.array([((pair_core == c) & (seg == s)).sum()
                       for c in range(NCORES)])
        nb = max(1, int(np.ceil(ns.max() / 128.0)))
        stream_blocks.append(nb)
    total_nb = sum(stream_blocks)
    pad_round = int(np.ceil(total_nb / 8.0)) * 8 - total_nb
    stream_blocks[0] += pad_round  # round total to a multiple of 8

    NBT = sum(stream_blocks)
    per_core = []
    for c in range(NCORES):
        vals = {k: np.zeros(NBT * 128, np.float32)
                for k in ("y0", "rc", "x0", "dx", "sm", "gy", "so", "rl")}
        off = 0
        for s in range(NSTREAM):
            nb = stream_blocks[s]
            if nb == 0:
                continue
            idx = np.nonzero((pair_core == c) & (seg == s))[0]
            n = len(idx)
            sl = slice(off * 128, off * 128 + n)
            pe = pair_edge[idx]
            vals["y0"][sl] = y0[pe]
            vals["rc"][sl] = recip[pe]
            vals["x0"][sl] = x0[pe]
            vals["dx"][sl] = dxe[pe]
            vals["sm"][sl] = sm[pe]
            vals["gy"][sl] = pair_row[idx].astype(np.float32)
            vals["so"][sl] = np.float32(64.0) * s
            vals["rl"][sl] = row_loc[pair_row[idx]].astype(np.float32)
            off += nb
        data = {k: np.ascontiguousarray(v.reshape(NBT, 128).T)
                for k, v in vals.items()}
        per_core.append(data)
    return per_core, core_rows, tuple(stream_blocks)


def _build_program(stream_blocks, repeats=1):
    key = (stream_blocks, repeats)
    if key in _prog_cache:
        return _prog_cache[key]
    NBT = sum(stream_blocks)
    nc = bacc.Bacc("TRN2", target_bir_lowering=False, debug=False,
                   num_devices=NCORES)

    ins = {}
    for name in ("y0", "rc", "x0", "dx", "sm", "gy", "so", "rl"):
        ins[name] = nc.dram_tensor(name, [128, NBT], DT, kind="ExternalInput")
    cbd = nc.dram_tensor("colorb", [RPC, 4], DT, kind="ExternalInput")
    outd = nc.dram_tensor("rgba", [RPC, W * 4], DT, kind="ExternalOutput")

    k128 = nc.inline_tensor(np.ascontiguousarray(
        np.broadcast_to(np.arange(128, dtype=np.float32), (128, 128))),
        name="k128const")
    r64 = nc.inline_tensor(np.ascontiguousarray(
        np.broadcast_to(np.arange(RPC, dtype=np.float32), (128, RPC))),
        name="r64const")

    import contextlib

    with tile.TileContext(nc) as tc:
        with (
            tc.tile_pool(name="const", bufs=1) as cpool,
            tc.tile_pool(name="sig", bufs=4) as sigpool,
            tc.tile_pool(name="w2", bufs=4) as w2pool,
            tc.tile_pool(name="psum", bufs=1, space="PSUM") as pspool,
            (tc.For_i(0, repeats, 1) if repeats > 1
             else contextlib.nullcontext()),
        ):
            k128t = cpool.tile([128, 128], DT)
            nc.sync.dma_start(k128t[:], k128[:])
            r64t = cpool.tile([128, RPC], DT)
            nc.sync.dma_start(r64t[:], r64[:])
            cbt = cpool.tile([RPC, 4], DT)
            nc.sync.dma_start(cbt[:], cbd[:])
            tin = {}
            for name in ("y0", "rc", "x0", "dx", "sm", "gy", "so", "rl"):
                tin[name] = cpool.tile([128, NBT], DT, name=f"in_{name}")
                nc.sync.dma_start(tin[name][:], ins[name][:])

            # t = (gy - y0) * recip;  bias = clamp(x0 + t * dx) - so
            tt = cpool.tile([128, NBT], DT)
            nc.vector.tensor_sub(tt[:], tin["gy"][:], tin["y0"][:])
            nc.vector.tensor_mul(tt[:], tt[:], tin["rc"][:])
            xct = cpool.tile([128, NBT], DT)
            nc.vector.tensor_mul(xct[:], tt[:], tin["dx"][:])
            nc.vector.tensor_add(xct[:], xct[:], tin["x0"][:])
            nc.vector.tensor_scalar_min(xct[:], xct[:], CLAMP_X)
            nc.vector.tensor_scalar_max(xct[:], xct[:], -CLAMP_X)
            nc.vector.tensor_sub(xct[:], xct[:], tin["so"][:])

            # coeff = sigmoid(20 t) * sigmoid(20 - 20 t) * sm
            b20 = cpool.tile([128, 1], DT)
            nc.vector.memset(b20[:], 20.0)
            tcl = cpool.tile([128, NBT], DT)
            nc.vector.tensor_scalar_min(tcl[:], tt[:], CLAMP_T)
            nc.vector.tensor_scalar_max(tcl[:], tcl[:], -CLAMP_T)
            v1 = cpool.tile([128, NBT], DT)
            nc.scalar.activation(v1[:], tcl[:], AF.Sigmoid, bias=0.0, scale=20.0)
            v2 = cpool.tile([128, NBT], DT)
            nc.scalar.activation(v2[:], tcl[:], AF.Sigmoid, bias=b20[:],
                                 scale=-20.0)
            cft = cpool.tile([128, NBT], DT)
            nc.vector.tensor_mul(cft[:], v1[:], v2[:])
            nc.vector.tensor_mul(cft[:], cft[:], tin["sm"][:])

            # LMASK[p, b] = [so_p >= 64 (b+1)]   b in [0, 7); col 7 pads to 0
            lmt = cpool.tile([128, NBT * NSTREAM], DT)
            lm3 = lmt[:].rearrange("p (j b) -> p j b", b=NSTREAM)
            nc.vector.memset(lmt[:], 0.0)
            for b in range(NSTREAM - 1):
                nc.vector.tensor_scalar(
                    lm3[:, :, b], tin["so"][:], float(64.0 * (b + 1)),
                    None, mybir.AluOpType.is_ge)

            psw = pspool.tile([RPC, NSTREAM * 128], DT)
            psl = pspool.tile([RPC, NSTREAM], DT)
            j = 0
            for s in range(NSTREAM):
                for js in range(stream_blocks[s]):
                    w2 = w2pool.tile([128, RPC], DT)
                    nc.vector.tensor_scalar(
                        w2[:], r64t[:], tin["rl"][:, j:j + 1],
                        cft[:, j:j + 1], mybir.AluOpType.is_equal,
                        mybir.AluOpType.mult)
                    sig = sigpool.tile([128, 128], DT)
                    nc.scalar.activation(sig[:], k128t[:], AF.Sigmoid,
                                         bias=xct[:, j:j + 1], scale=-1.0)
                    nc.tensor.matmul(psw[:, s * 128:(s + 1) * 128], w2[:],
                                     sig[:], start=(js == 0),
                                     stop=(js == stream_blocks[s] - 1))
                    nc.tensor.matmul(
                        psl[:], w2[:],
                        lmt[:, j * NSTREAM:(j + 1) * NSTREAM],
                        start=(j == 0), stop=(j == NBT - 1))
                    j += 1

            # fold stream slices + left constants into wind[64, 512]
            pslc = cpool.tile([RPC, NSTREAM], DT)
            nc.vector.tensor_copy(pslc[:], psl[:])
            wsb = cpool.tile([RPC, NSTREAM * 128], DT)
            nc.vector.tensor_copy(wsb[:], psw[:])
            wind = cpool.tile([RPC, W], DT)
            nc.vector.tensor_scalar_add(wind[:, 0:64], wsb[:, 0:64],
                                        pslc[:, 0:1])
            for b in range(1, NSTREAM):
                lo = wsb[:, (b - 1) * 128 + 64:(b - 1) * 128 + 128]
                hi = wsb[:, b * 128:b * 128 + 64]
                dst = wind[:, b * 64:(b + 1) * 64]
                if b < NSTREAM - 1:
                    nc.vector.scalar_tensor_tensor(
                        dst, lo, pslc[:, b:b + 1], hi,
                        op0=mybir.AluOpType.add, op1=mybir.AluOpType.add)
                else:
                    nc.vector.tensor_add(dst, lo, hi)

            rgba = cpool.tile([RPC, W * 4], DT)
            for ch in range(3):
                nc.vector.tensor_copy(
                    rgba[:, ch::4],
                    cbt[:, ch:ch + 1].broadcast_to((RPC, W)))
            nc.scalar.activation(rgba[:, 3::4], wind[:], AF.Sigmoid,
                                 bias=0.0, scale=4.0)
            nc.sync.dma_start(outd[:], rgba[:])

    nc.compile()
    _prog_cache[key] = nc
    return nc


def _in_maps(per_core, color):
    cb = np.zeros((RPC, 4), np.float32)
    cb[:, :3] = np.asarray(color, np.float32)[None, :]
    maps = []
    for c in range(NCORES):
        m = dict(per_core[c])
        m["colorb"] = cb
        maps.append(m)
    return maps


def kernel(control_points, color):
    per_core, core_rows, stream_blocks = _host_prep(control_points)
    nc = _build_program(stream_blocks)
    res = run_bass_kernel_spmd(nc, _in_maps(per_core, color),
                               list(range(NCORES)))
    out = np.empty((H, W, 4), np.float32)
    for c in range(NCORES):
        rg = res.results[c]["rgba"].reshape(RPC, W, 4)
        out[np.asarray(core_rows[c], np.int64)] = rg
    return out


# revision 12
# speedup vs baseline: 1.4118x; 1.4118x over previous
"""Differentiable Bezier path renderer on 8 Trainium2 NeuronCores.

Strategy
--------
The reference rasterizes M=2048 path edges into a 512x512 soft
winding-number image:

    wind[h, w] = sum_e coeff(e, h) * sigmoid(x_cross(e, h) - w)
    coeff(e,h) = sigmoid(20 t) * sigmoid(20 (1 - t)) * sign(dy_e) * [|dy_e|>=1e-6]
    t          = (h - y0_e) / (dy_e + 1e-8),  x_cross = x0_e + t * dx_e

Two sparsity facts carry the kernel:
  * coeff is negligible (< 5e-8) outside t in [-0.85, 1.85], so only
    ~55k of the 1M (edge, row) pairs matter.
  * sigmoid(x_cross - w) saturates outside |x_cross - w| <= 18, so per
    pair only a 36px-wide transition window needs real sigmoids; the
    whole region left of the window contributes exactly coeff.

The host enumerates active pairs, assigns rows to cores so every core
gets an equal pair load (64 rows per core, no collectives needed), and
packs pairs into blocks of 128 "slots".  Each pair gets one 128-wide,
64-aligned window segment s (columns [64 s, 64 s + 128)) that is
guaranteed to contain its transition; blocks group pairs of the same s.

Per block the device computes, slots on the partition axis:
  * ScalarE : SIGW[p, k] = sigmoid((xc_p - 64 s) - k), k in [0,128)
  * VectorE : W2[p, r] = (iota_r == row_p) * coeff_p     (fused is_eq*mul)
  * TensorE : PSW[r, s-slice] += W2.T @ SIGW             (window part)
              PSL[r, b]       += W2.T @ LMASK            (saturated part,
                LMASK[p, b] = [64 (b+1) <= 64 s_p], b in [0,7))
Afterwards VectorE folds the 8 overlapping stream slices plus the
broadcast left-constants into wind[64, 512], and ScalarE writes
alpha = sigmoid(4 wind) into an interleaved RGBA tile (rgb = broadcast
input color).  The host only gathers per-edge scalars per pair and
reassembles the 8 per-core row sets.
"""

import numpy as np

import concourse.bacc as bacc
import concourse.mybir as mybir
import concourse.tile as tile
from concourse.bass_utils import run_bass_kernel_spmd

H = 512
W = 512
S = 64          # cubic bezier segments
TSAMP = 32      # samples per segment
M = S * TSAMP   # path points == edges
NCORES = 8
RPC = H // NCORES  # rows per core
NSTREAM = 8        # 64-aligned window segment streams
WIN = 18.0         # sigmoid saturation half-width (sigmoid(-18) ~ 1.5e-8)
TB = np.float32(0.85)     # t-window bound: sigmoid(-17) ~ 4.1e-8
CLAMP_T = 60.0            # |20 t| <= 1200, keeps ACT inputs finite
CLAMP_X = 10000.0         # sigmoid saturated way before +-CLAMP_X
DT = mybir.dt.float32
AF = mybir.ActivationFunctionType
PNAMES = ("y0", "rc", "x0", "dx", "sm", "gy", "so", "rl")

_prog_cache = {}


def _host_prep(control_points):
    """Sample the path, enumerate active (edge, row) pairs, assign rows to
    cores, pack pairs into per-stream blocks of 128 slots.

    Returns (per_core_inputs, core_rows, stream_blocks) where
    stream_blocks[s] is the number of blocks of stream s (same for all
    cores; short cores are padded with coeff=0 slots)."""
    cp = np.asarray(control_points, dtype=np.float32)
    p0 = cp[0:3 * S:3][:, None, :]
    p1 = cp[1:3 * S:3][:, None, :]
    p2 = cp[2:3 * S:3][:, None, :]
    p3 = cp[3:3 * S + 1:3][:, None, :]
    t = np.linspace(0.0, 1.0, TSAMP, dtype=np.float32)[None, :, None]
    mt = np.float32(1.0) - t
    pts = (mt ** 3) * p0 + 3.0 * (mt ** 2) * t * p1 \
        + 3.0 * mt * (t ** 2) * p2 + (t ** 3) * p3
    path = pts.reshape(-1, 2).astype(np.float32)

    nxt = np.roll(path, -1, axis=0)
    x0 = path[:, 0]
    y0 = path[:, 1]
    dy = nxt[:, 1] - y0
    dxe = nxt[:, 0] - x0
    dys = (dy + np.float32(1e-8)).astype(np.float32)
    recip = (np.float32(1.0) / dys).astype(np.float32)
    sm = (np.sign(dy) * (np.abs(dy) >= np.float32(1e-6))).astype(np.float32)

    g1 = y0 + (-TB) * dys
    g2 = y0 + (np.float32(1.0) + TB) * dys
    rlo = np.maximum(np.ceil(np.minimum(g1, g2)), 0.0).astype(np.int64)
    rhi = np.minimum(np.floor(np.maximum(g1, g2)), H - 1).astype(np.int64)
    act = (sm != 0) & (rhi >= rlo)
    eact = np.nonzero(act)[0]
    counts = (rhi[eact] - rlo[eact] + 1).astype(np.int64)
    pair_edge = np.repeat(eact, counts)
    pair_row = np.concatenate(
        [np.arange(rlo[e], rhi[e] + 1, dtype=np.int64) for e in eact]
    ) if len(eact) else np.zeros(0, np.int64)

    # Window segment per pair, from host-side x_cross (the ~1 ulp
    # host/device difference is covered by the 64 - 36 px fit margin).
    tval = ((pair_row.astype(np.float32) - y0[pair_edge]) * recip[pair_edge])
    xcv = x0[pair_edge] + tval * dxe[pair_edge]
    xcv = np.clip(xcv, -CLAMP_X, CLAMP_X)
    seg = np.clip(np.floor((xcv - WIN) / 64.0), 0, NSTREAM - 1).astype(np.int64)

    # Balanced row -> core assignment (equal pair load, RPC rows per core).
    rowcnt = np.bincount(pair_row, minlength=H)
    order = np.argsort(-rowcnt, kind="stable")
    core_rows = [[] for _ in range(NCORES)]
    loads = np.zeros(NCORES, np.int64)
    for r in order:
        avail = [c for c in range(NCORES) if len(core_rows[c]) < RPC]
        c = min(avail, key=lambda i: loads[i])
        core_rows[c].append(int(r))
        loads[c] += rowcnt[r]
    row_core = np.empty(H, np.int64)
    row_loc = np.empty(H, np.int64)
    for c in range(NCORES):
        for i, r in enumerate(core_rows[c]):
            row_core[r] = c
            row_loc[r] = i

    pair_core = row_core[pair_row]
    # blocks per stream = max over cores (SPMD: one program for all cores),
    # rounded up so near-identical inputs reuse the compiled program.
    stream_blocks = []
    for s in range(NSTREAM):
        ns = np.array([((pair_core == c) & (seg == s)).sum()
                       for c in range(NCORES)])
        nb = max(1, int(np.ceil(ns.max() / 128.0)))
        stream_blocks.append(nb)
    total_nb = sum(stream_blocks)
    pad_round = int(np.ceil(total_nb / 8.0)) * 8 - total_nb
    stream_blocks[0] += pad_round  # round total to a multiple of 8

    NBT = sum(stream_blocks)
    per_core = []
    for c in range(NCORES):
        vals = {k: np.zeros(NBT * 128, np.float32) for k in PNAMES}
        off = 0
        for s in range(NSTREAM):
            nb = stream_blocks[s]
            if nb == 0:
                continue
            idx = np.nonzero((pair_core == c) & (seg == s))[0]
            n = len(idx)
            sl = slice(off * 128, off * 128 + n)
            pe = pair_edge[idx]
            vals["y0"][sl] = y0[pe]
            vals["rc"][sl] = recip[pe]
            vals["x0"][sl] = x0[pe]
            vals["dx"][sl] = dxe[pe]
            vals["sm"][sl] = sm[pe]
            vals["gy"][sl] = pair_row[idx].astype(np.float32)
            vals["so"][sl] = np.float32(64.0) * s
            vals["rl"][sl] = row_loc[pair_row[idx]].astype(np.float32)
            off += nb
        packed = np.concatenate(
            [vals[k].reshape(NBT, 128).T for k in PNAMES] +
            [np.zeros((128, 4), np.float32)], axis=1)
        per_core.append({"params": np.ascontiguousarray(packed)})
    return per_core, core_rows, tuple(stream_blocks)


def _build_program(stream_blocks, repeats=1):
    key = (stream_blocks, repeats)
    if key in _prog_cache:
        return _prog_cache[key]
    NBT = sum(stream_blocks)
    nc = bacc.Bacc("TRN2", target_bir_lowering=False, debug=False,
                   num_devices=NCORES)

    npar = len(PNAMES) * NBT + 4
    pard = nc.dram_tensor("params", [128, npar], DT, kind="ExternalInput")
    outd = nc.dram_tensor("rgba", [RPC, W * 4], DT, kind="ExternalOutput")

    cst = np.zeros((128, 128 + RPC), np.float32)
    cst[:, :128] = np.arange(128, dtype=np.float32)[None, :]
    cst[:, 128:] = np.arange(RPC, dtype=np.float32)[None, :]
    cstd = nc.inline_tensor(np.ascontiguousarray(cst), name="cstconst")

    import contextlib

    with tile.TileContext(nc) as tc:
        with (
            tc.tile_pool(name="const", bufs=1) as cpool,
            tc.tile_pool(name="sig", bufs=4) as sigpool,
            tc.tile_pool(name="w2", bufs=4) as w2pool,
            tc.tile_pool(name="psum", bufs=1, space="PSUM") as pspool,
            (tc.For_i(0, repeats, 1) if repeats > 1
             else contextlib.nullcontext()),
        ):
            cstt = cpool.tile([128, 128 + RPC], DT)
            nc.sync.dma_start(cstt[:], cstd[:])
            k128t = cstt[:, 0:128]
            r64t = cstt[:, 128:128 + RPC]
            part = cpool.tile([128, npar], DT)
            nc.sync.dma_start(part[:], pard[:])
            cbt = part[0:RPC, len(PNAMES) * NBT:len(PNAMES) * NBT + 4]
            tin = {n: part[:, i * NBT:(i + 1) * NBT]
                   for i, n in enumerate(PNAMES)}

            # t = (gy - y0) * recip;  bias = clamp(x0 + t * dx) - so
            tt = cpool.tile([128, NBT], DT)
            nc.vector.tensor_sub(tt[:], tin["gy"], tin["y0"])
            nc.vector.tensor_mul(tt[:], tt[:], tin["rc"])
            xct = cpool.tile([128, NBT], DT)
            nc.vector.tensor_mul(xct[:], tt[:], tin["dx"])
            nc.vector.tensor_add(xct[:], xct[:], tin["x0"])
            nc.vector.tensor_scalar_min(xct[:], xct[:], CLAMP_X)
            nc.vector.tensor_scalar_max(xct[:], xct[:], -CLAMP_X)
            nc.vector.tensor_sub(xct[:], xct[:], tin["so"])

            # coeff = sigmoid(20 t) * sigmoid(20 - 20 t) * sm
            b20 = cpool.tile([128, 1], DT)
            nc.vector.memset(b20[:], 20.0)
            tcl = cpool.tile([128, NBT], DT)
            nc.vector.tensor_scalar_min(tcl[:], tt[:], CLAMP_T)
            nc.vector.tensor_scalar_max(tcl[:], tcl[:], -CLAMP_T)
            v1 = cpool.tile([128, NBT], DT)
            nc.scalar.activation(v1[:], tcl[:], AF.Sigmoid, bias=0.0, scale=20.0)
            v2 = cpool.tile([128, NBT], DT)
            nc.scalar.activation(v2[:], tcl[:], AF.Sigmoid, bias=b20[:],
                                 scale=-20.0)
            cft = cpool.tile([128, NBT], DT)
            nc.vector.tensor_mul(cft[:], v1[:], v2[:])
            nc.vector.tensor_mul(cft[:], cft[:], tin["sm"])

            # LMASK[p, b] = [so_p >= 64 (b+1)]   b in [0, 7); col 7 pads to 0
            lmt = cpool.tile([128, NBT * NSTREAM], DT)
            lm3 = lmt[:].rearrange("p (j b) -> p j b", b=NSTREAM)
            nc.vector.memset(lmt[:], 0.0)
            for b in range(NSTREAM - 1):
                nc.vector.tensor_scalar(
                    lm3[:, :, b], tin["so"], float(64.0 * (b + 1)),
                    None, mybir.AluOpType.is_ge)

            psw = pspool.tile([RPC, NSTREAM * 128], DT)
            psl = pspool.tile([RPC, NSTREAM], DT)
            j = 0
            for s in range(NSTREAM):
                for js in range(stream_blocks[s]):
                    w2 = w2pool.tile([128, RPC], DT)
                    nc.vector.tensor_scalar(
                        w2[:], r64t, tin["rl"][:, j:j + 1],
                        cft[:, j:j + 1], mybir.AluOpType.is_equal,
                        mybir.AluOpType.mult)
                    sig = sigpool.tile([128, 128], DT)
                    nc.scalar.activation(sig[:], k128t, AF.Sigmoid,
                                         bias=xct[:, j:j + 1], scale=-1.0)
                    nc.tensor.matmul(psw[:, s * 128:(s + 1) * 128], w2[:],
                                     sig[:], start=(js == 0),
                                     stop=(js == stream_blocks[s] - 1))
                    nc.tensor.matmul(
                        psl[:], w2[:],
                        lmt[:, j * NSTREAM:(j + 1) * NSTREAM],
                        start=(j == 0), stop=(j == NBT - 1))
                    j += 1

            # fold stream slices + left constants into wind[64, 512]
            pslc = cpool.tile([RPC, NSTREAM], DT)
            nc.vector.tensor_copy(pslc[:], psl[:])
            wsb = cpool.tile([RPC, NSTREAM * 128], DT)
            nc.vector.tensor_copy(wsb[:], psw[:])
            wind = cpool.tile([RPC, W], DT)
            nc.vector.tensor_scalar_add(wind[:, 0:64], wsb[:, 0:64],
                                        pslc[:, 0:1])
            for b in range(1, NSTREAM):
                lo = wsb[:, (b - 1) * 128 + 64:(b - 1) * 128 + 128]
                hi = wsb[:, b * 128:b * 128 + 64]
                dst = wind[:, b * 64:(b + 1) * 64]
                if b < NSTREAM - 1:
                    nc.vector.scalar_tensor_tensor(
                        dst, lo, pslc[:, b:b + 1], hi,
                        op0=mybir.AluOpType.add, op1=mybir.AluOpType.add)
                else:
                    nc.vector.tensor_add(dst, lo, hi)

            rgba = cpool.tile([RPC, W * 4], DT)
            for ch in range(3):
                nc.vector.tensor_copy(
                    rgba[:, ch::4],
                    cbt[:, ch:ch + 1].broadcast_to((RPC, W)))
            nc.scalar.activation(rgba[:, 3::4], wind[:], AF.Sigmoid,
                                 bias=0.0, scale=4.0)
            nc.sync.dma_start(outd[:], rgba[:])

    nc.compile()
    _prog_cache[key] = nc
    return nc


def _in_maps(per_core, color):
    maps = []
    for c in range(NCORES):
        p = per_core[c]["params"].copy()
        p[:RPC, -4:-1] = np.asarray(color, np.float32)[None, :]
        maps.append({"params": p})
    return maps


def kernel(control_points, color):
    per_core, core_rows, stream_blocks = _host_prep(control_points)
    nc = _build_program(stream_blocks)
    res = run_bass_kernel_spmd(nc, _in_maps(per_core, color),
                               list(range(NCORES)))
    out = np.empty((H, W, 4), np.float32)
    for c in range(NCORES):
        rg = res.results[c]["rgba"].reshape(RPC, W, 4)
        out[np.asarray(core_rows[c], np.int64)] = rg
    return out
